# revision 41
# baseline (speedup 1.0000x reference)
"""Contrastive loss (SimCLR-style NT-Xent) Trainium2 kernel, symmetric-GEMM.

Full inputs z1, z2: [4096, 1024] f32. Output: scalar f32 loss.

sim = reps @ reps.T is symmetric, and so is exp(sim). The loss needs only
per-row sums of exp(sim) plus two diagonals, so it suffices to compute each
unordered 128x128 block-pair ONCE (2080 of 4096 blocks = 50.8% of the full
GEMM, which bounds the fp8 PE time) and recover the lower-triangle row sums
as COLUMN sums of the computed blocks.

Decomposition (8 cores, SPMD-uniform, no collectives):
  - 64x64 grid of 128x128 blocks. Core k owns row blocks r = k + 8m
    (m = 0..7) and computes blocks (r, (r+d) mod 64) for d = 0..31, plus
    d = 32 only for r < 32. Every unordered pair is covered exactly once
    and every core runs an IDENTICAL 260-block program.
  - Per-core inputs are pre-rotated by k blocks on the host, so the
    rotated column index (8m + d) mod 64 is core-independent. The d=0
    (self) block sits at rotated column block 8m; the d=32 (positive
    pair) block for m' = q-4 sits at rotated column block 8q.
  - Group the 32 window tiles by q = (m + t) mod 8 (t = tile index 0..3
    within a row's 32-block window): group q's four [128, 1024] tiles all
    read B chunk-pair q and all write the SAME rotated accumulator octet
    [1024q, 1024q+1024). Each B chunk is consumed by exactly one group
    (streamed, then dead) and each accumulator octet is complete when its
    group ends, so its DMA-out overlaps the next group's compute.
  - Per tile: 8 fp8 DoubleRow matmuls (K=256 each) -> ACT exp(s*x - 10)
    with fused per-row accumulation (accum_out) -> row sums; DVE/Pool adds
    the exp tile into the column accumulator (engine alternates with group
    parity to halve per-engine load). t=0 tiles extract the raw self-diag
    from PSUM (identity mul + reduce) and exclude the self block's 128
    columns from the column add. Groups q >= 4 append the d=32 positive
    block for m = q-4: 4 matmuls, exp+accum, raw diag extract, column add.
  - Host: S_row (row sums) from `sums`, C_col from partition-reducing the
    un-rotated accumulators, then per row i:
      T = S_row + C_col - exp(s*dself - 10) + exp(s*draw - 10)
      loss_row = (10 + ln(T)) - (s*draw)    [s*draw = scaled positive]
    mean over 8192 rows. A few K flops in f64.
"""

import time
from contextlib import ExitStack

import numpy as np
import ml_dtypes

import concourse.bass as bass
import concourse.tile as tile
from concourse import bacc
from concourse import mybir
from concourse import bass_utils
from concourse.masks import make_identity

B = 4096
D = 1024
S = 2 * B  # 8192 rows/cols of sim
NCORES = 8
P = 128
NB = S // P  # 64 block grid
M_TILES = 8  # row blocks per core; global row block = k + 8m
T_TILES = 4  # [128, 1024] tiles per row window (d = 0..31)
N_TILE = 1024
KT = D // P  # 8 k-tiles
N_HALF = 512  # max matmul moving free dim into one PSUM bank
INV_T = 10.0  # 1 / temperature
EPS = 1e-12
FP8_SCALE = 256.0  # input scale: keeps fp8e4m3 operands in their sweet spot
SIM_SCALE = INV_T / (FP8_SCALE * FP8_SCALE)  # exp(SIM_SCALE * raw - INV_T)
SUM_COLS = 36  # 32 window tiles + 4 d=32 blocks
DIAG_COLS = 12  # 8 self + 4 positive

_FP32 = mybir.dt.float32
_BF16 = mybir.dt.bfloat16
_FP8 = mybir.dt.float8e4
_FP8_NP = mybir.dt.np(_FP8)


def _build_bass():
    # Bacc (not raw Bass): its compile() runs generate_event_semaphores,
    # which splits multi-semaphore waits into standalone EventSemaphore
    # instructions — engine instructions can encode only one wait.
    nc = bacc.Bacc("TRN2", debug=False, num_devices=NCORES, enable_partition_id=False)
    # lhsT blocked per m on the host: [m, p, kt, col]; m-block = global row
    # block k+8m. p-major layout keeps each partition's read contiguous
    # (1KB runs), so the DMA issue is cheap (~0.65us vs ~2-3us strided).
    lhsT = nc.dram_tensor(
        "lhst", [M_TILES, P, KT, P], _FP8, kind="ExternalInput"
    ).ap()
    # brot blocked per 512-column chunk, rotated by k blocks on the host:
    # [chunk, p, kt, col]; chunk-pair q feeds exactly group q.
    brot = nc.dram_tensor(
        "brot", [S // N_HALF, P, KT, N_HALF], _FP8, kind="ExternalInput"
    ).ap()
    # Raw reductions out; the tiny final combine runs on the host, which
    # avoids a 1.3us ACT table switch (Ln) in the device tail.
    sums_out = nc.dram_tensor("sums", [P, SUM_COLS], _FP32, kind="ExternalOutput").ap()
    diag_out = nc.dram_tensor("diag", [P, DIAG_COLS], _FP32, kind="ExternalOutput").ap()
    # Column accumulator (rotated space); host partition-reduces it. bf16:
    # halves DVE add time (16-bit 2x) and the DMA-out; the ~0.2% partial-sum
    # rounding is far inside the fp8-GEMM noise floor. Octet 7 (the last
    # group) bypasses the accumulator: its raw exp tiles ship to etail /
    # e32tail as they land, so the device tail has no add chain.
    acc_out = nc.dram_tensor(
        "accout", [P, 7 * N_TILE], _BF16, kind="ExternalOutput"
    ).ap()
    etail_out = nc.dram_tensor(
        "etail", [T_TILES, P, N_TILE], _BF16, kind="ExternalOutput"
    ).ap()
    e32tail_out = nc.dram_tensor(
        "e32tail", [P, P], _BF16, kind="ExternalOutput"
    ).ap()

    # Pre-TileContext const region (same pattern as Bass.__init__'s
    # const_aps): values read by hot-loop instructions with no tracked
    # dependency, so they add no per-instruction sync waits. Hand off with
    # one semaphore to the only consumers (ACT reads the bias const, DVE
    # the identity).
    bias_th = nc.alloc_sbuf_tensor("const-f32-neg10", [P, 1], _FP32)
    nc.gpsimd.memset(bias_th.ap(), -INV_T)
    nc.const_aps.aps[(_FP32, -INV_T)] = bias_th.ap()
    ident_th = nc.alloc_sbuf_tensor("identity-f32", [P, P], _FP32)
    nc.gpsimd.memset(ident_th.ap(), 0.0)
    ident_inst = nc.gpsimd.affine_select(
        out=ident_th.ap(),
        in_=ident_th.ap(),
        compare_op=mybir.AluOpType.not_equal,
        fill=1.0,
        base=0,
        pattern=[[-1, P]],
        channel_multiplier=1,
    )
    const_sem = nc.alloc_semaphore("const-ready")
    ident_inst.then_inc(const_sem, 1)
    nc.vector.wait_ge(const_sem, 1)
    nc.scalar.wait_ge(const_sem, 1)

    with tile.TileContext(nc) as tc:
        _body(
            tc, lhsT, brot, sums_out, diag_out, acc_out, etail_out, e32tail_out,
            ident_th.ap(),
        )
    nc.compile()
    return nc


def _body(tc, lhsT, brot, sums_out, diag_out, acc_out, etail_out, e32tail_out, ident):
    nc = tc.nc
    AF = mybir.ActivationFunctionType

    a_view = lhsT.rearrange("m p k c -> p m k c")  # [128, 8, 8, 128]

    ctx = ExitStack()
    singles = ctx.enter_context(tc.tile_pool(name="singles", bufs=1))
    # 3 tiles x 2 banks + 1 bank for d=32: deep PSUM pipeline so matmuls
    # never wait on the ACT exp/read-accumulator chain of a recycled tile.
    pspool = ctx.enter_context(tc.tile_pool(name="psum", bufs=3, space="PSUM"))
    ps32pool = ctx.enter_context(tc.tile_pool(name="psum32", bufs=1, space="PSUM"))
    # Exp tiles feed the column-accumulator adds; f32 keeps the adds
    # single-dtype.
    epool = ctx.enter_context(tc.tile_pool(name="exps", bufs=6))
    e32pool = ctx.enter_context(tc.tile_pool(name="exps32", bufs=2))
    scratch = ctx.enter_context(tc.tile_pool(name="scratch", bufs=12))

    # Resident operands, chunk-major so every DMA is contiguous per
    # partition on BOTH sides (cheap descriptors, fast transfers):
    # a_t[p, m, kt, c], b_t[p, chunk, kt, c]. A 1024-tile's half reads
    # exactly one chunk, so matmul APs stay simple [2, cols] patterns.
    a_t = singles.tile([P, M_TILES, KT, P], _FP8)
    b_t = singles.tile([P, S // N_HALF, KT, N_HALF], _FP8)
    # Column accumulator in rotated space (bf16: 2x DVE rate).
    acc = singles.tile([P, S], _BF16)
    # Per-row partial sums: col 4m+t for window tiles, 32+m for d=32.
    sums = singles.tile([P, SUM_COLS], _FP32)
    # Raw (pre-exp, scaled) diagonals: cols [0:8] self (by q=m), [8:12] pos.
    diag = singles.tile([P, DIAG_COLS], _FP32)

    def load_a(m):
        nc.sync.dma_start(out=a_t[:, m, :, :], in_=a_view[:, m, :, :])

    def load_b(c, kt0=0, kt1=KT):
        nc.sync.dma_start(
            out=b_t[:, c, kt0:kt1, :],
            in_=brot[c][:, kt0:kt1, :],
        )

    # a0 alone rides the Activation hwdge queue (one cheap issue, done well
    # before the first ACTIVATE); everything else is on Sync, interleaved in
    # consumption order. Chunks 0/1 are split by kt-halves so the first
    # matmuls gate on fewer bytes.
    nc.scalar.dma_start(out=a_t[:, 0, :, :], in_=a_view[:, 0, :, :])
    load_b(0, 0, 4)
    load_b(0, 4, 8)
    load_b(1, 0, 4)
    load_b(1, 4, 8)
    for m in (7, 6, 5):
        load_a(m)
    load_b(2)
    load_b(3)
    load_a(1)
    load_a(2)
    load_b(4)
    load_b(5)
    load_a(3)
    load_a(4)
    for c in range(6, S // N_HALF):
        load_b(c)

    # Zero the accumulator on the (otherwise idle) gpsimd engine, one memset
    # per octet so group q's first add only waits on its own octet. Octet 7
    # never touches the accumulator (raw-exp tail path).
    for q in range(7):
        nc.gpsimd.memset(acc[:, q * N_TILE : (q + 1) * N_TILE], 0.0)

    # Warm the PE's p-state while the first B chunk is still in flight:
    # f32 matmuls on the (untracked, possibly-garbage) identity const,
    # results discarded. Continuous execution ramps the clock 0.65 -> 2.4
    # GHz, so the first real matmuls start near full speed.
    warm = ps32pool.tile([P, N_HALF], _FP32)
    for _ in range(10):
        nc.tensor.matmul(warm[:, 0:64], ident, ident[:, 0:64], start=True, stop=True)

    def extract_diag(src_ap, dst_col):
        diag_t = scratch.tile([P, P], _FP32)
        nc.vector.tensor_mul(diag_t, src_ap, ident)
        nc.vector.reduce_sum(
            diag[:, dst_col : dst_col + 1], diag_t, axis=mybir.AxisListType.X
        )

    for q in range(8):
        c0 = q * N_TILE
        if q >= 4:
            # d=32 positive-pair block for m = q-4 lands on rotated column
            # block 8q — the first block of this group's octet. Processed
            # FIRST so its exp/extract/add never trail the group's big-tile
            # chain (keeps the q=7 tail short).
            md = q - 4
            ps2 = ps32pool.tile([P, N_HALF], _FP32)
            for kt in range(0, KT, 2):
                nc.tensor.matmul(
                    ps2[:, 0:P],
                    a_t[:, md, kt : kt + 2, :],
                    b_t[:, 2 * q, kt : kt + 2, 0:P],
                    start=(kt == 0),
                    stop=(kt == KT - 2),
                    perf_mode=mybir.MatmulPerfMode.DoubleRow,
                )
            e32 = e32pool.tile([P, P], _BF16)
            nc.scalar.activation(
                out=e32,
                in_=ps2[:, 0:P],
                func=AF.Exp,
                bias=-INV_T,
                scale=SIM_SCALE,
                accum_out=sums[:, 32 + md : 33 + md],
            )
            extract_diag(ps2[:, 0:P], 8 + md)
            if q == 7:
                nc.sync.dma_start(out=e32tail_out, in_=e32)
            else:
                nc.vector.tensor_add(acc[:, c0 : c0 + P], acc[:, c0 : c0 + P], e32)
        for t in range(T_TILES):
            m = (q - t) % M_TILES
            ps = pspool.tile([P, N_TILE], _FP32)
            for half in range(2):
                hs = slice(half * N_HALF, (half + 1) * N_HALF)
                for kt in range(0, KT, 2):
                    nc.tensor.matmul(
                        ps[:, hs],
                        a_t[:, m, kt : kt + 2, :],
                        b_t[:, 2 * q + half, kt : kt + 2, :],
                        start=(kt == 0),
                        stop=(kt == KT - 2),
                        perf_mode=mybir.MatmulPerfMode.DoubleRow,
                    )
            e_t = epool.tile([P, N_TILE], _BF16)
            nc.scalar.activation(
                out=e_t,
                in_=ps,
                func=AF.Exp,
                bias=-INV_T,
                scale=SIM_SCALE,
                accum_out=sums[:, 4 * m + t : 4 * m + t + 1],
            )
            if t == 0:
                # Self block: raw diag out; its 128 columns are excluded
                # from the column sums (on-device slice / host skip).
                extract_diag(ps[:, 0:P], q)
            if q == 7:
                # Tail path: ship the raw exp tile as soon as it lands; the
                # host does octet 7's column accumulation. Kills the final
                # add chain + piece DMA from the device critical path.
                nc.sync.dma_start(out=etail_out[t], in_=e_t)
            elif t == 0:
                nc.vector.tensor_add(
                    acc[:, c0 + P : c0 + N_TILE],
                    acc[:, c0 + P : c0 + N_TILE],
                    e_t[:, P:N_TILE],
                )
            else:
                nc.vector.tensor_add(
                    acc[:, c0 : c0 + N_TILE], acc[:, c0 : c0 + N_TILE], e_t
                )
        if q < 7:
            # Octet q is final: ship it while the next group computes.
            nc.sync.dma_start(
                out=acc_out[:, c0 : c0 + N_TILE], in_=acc[:, c0 : c0 + N_TILE]
            )

    # Scalar queue: runs in parallel with the last acc piece on Sync.
    nc.scalar.dma_start(out=sums_out, in_=sums)
    nc.scalar.dma_start(out=diag_out, in_=diag)
    ctx.close()


_NC_CACHE = {}


def _get_nc():
    if "nc" not in _NC_CACHE:
        _NC_CACHE["nc"] = _build_bass()
    return _NC_CACHE["nc"]


def _make_in_maps(z1, z2):
    z1 = np.asarray(z1, dtype=np.float32)
    z2 = np.asarray(z2, dtype=np.float32)
    z = np.concatenate([z1, z2], axis=0)  # [8192, 1024]
    nrm = np.sqrt(np.sum(z * z, axis=1, keepdims=True, dtype=np.float32))
    n = z / np.maximum(nrm, EPS)
    repsT = np.ascontiguousarray(n.T * FP8_SCALE).astype(_FP8_NP)  # [1024, 8192]
    in_maps = []
    for c in range(NCORES):
        # Stationary rows: global row blocks c + 8m, blocked [m, p, kt, col]
        # (p-major so each partition's DMA read is one contiguous 1KB run).
        lhsT_blk = np.stack(
            [
                repsT[:, P * (c + 8 * m) : P * (c + 8 * m) + P]
                .reshape(KT, P, P)
                .transpose(1, 0, 2)
                for m in range(M_TILES)
            ]
        )
        # Moving columns rotated left by c blocks, blocked [chunk, p, kt, col].
        rolled = np.concatenate([repsT[:, P * c :], repsT[:, : P * c]], axis=1)
        b_blk = np.ascontiguousarray(
            rolled.reshape(KT, P, S // N_HALF, N_HALF).transpose(2, 1, 0, 3)
        )
        in_maps.append({"lhst": np.ascontiguousarray(lhsT_blk), "brot": b_blk})
    return in_maps


def _combine(results):
    # Per row i: T = S_row + C_col - e_self + e_pos;
    # loss_row = ln(T) - (SIM_SCALE*draw - INV_T). Host f64, a few M flops.
    S_row = np.zeros(S, dtype=np.float64)
    C_col = np.zeros(S, dtype=np.float64)
    dself = np.zeros(S, dtype=np.float64)
    draw = np.zeros(B, dtype=np.float64)
    for k, r in enumerate(results):
        sums = r["sums"].astype(np.float64)
        diag = r["diag"].astype(np.float64)
        for m in range(M_TILES):
            rows = slice(P * (k + 8 * m), P * (k + 8 * m) + P)
            S_row[rows] += sums[:, 4 * m : 4 * m + 4].sum(axis=1)
            if m < 4:
                S_row[rows] += sums[:, 32 + m]
                draw[rows] = diag[:, 8 + m]
            dself[rows] = diag[:, m]
        # Rotated-space column sums: octets 0..6 from the device accumulator,
        # octet 7 from the raw tail exp tiles (t0's first 128 cols are the
        # self block — excluded; e32tail covers the octet's first block).
        colsum = np.empty(S, dtype=np.float64)
        colsum[: 7 * 1024] = np.asarray(r["accout"], dtype=np.float64).sum(axis=0)
        etail = np.asarray(r["etail"], dtype=np.float64)  # [4, 128, 1024]
        oct7 = etail[1:].sum(axis=(0, 1))
        oct7[P:] += etail[0, :, P:].sum(axis=0)
        oct7[:P] += np.asarray(r["e32tail"], dtype=np.float64).sum(axis=0)
        colsum[7 * 1024 :] = oct7
        C_col += np.roll(colsum, P * k)  # rotated col c' -> global c' + 128k
    draw_full = np.concatenate([draw, draw])
    e_pos = np.exp(SIM_SCALE * draw_full - INV_T)
    e_self = np.exp(SIM_SCALE * dself - INV_T)
    T = S_row + C_col - e_self + e_pos
    loss = np.mean(np.log(T) + INV_T - SIM_SCALE * draw_full)
    return np.array(loss, dtype=np.float32)


def run_traced(z1, z2, **spmd_kwargs):
    """Run on HW with profiling; returns (loss, BassKernelResults)."""
    nc = _get_nc()
    in_maps = _make_in_maps(z1, z2)
    res = bass_utils.run_bass_kernel_spmd(
        nc, in_maps, core_ids=list(range(NCORES)), trace=True, **spmd_kwargs
    )
    return _combine(res.results), res


def kernel(z1, z2):
    nc = _get_nc()
    in_maps = _make_in_maps(z1, z2)
    last_err = None
    for _attempt in range(3):
        try:
            res = bass_utils.run_bass_kernel_spmd(
                nc, in_maps, core_ids=list(range(NCORES))
            )
            return _combine(res.results)
        except Exception as e:  # transient device wedge: retry
            last_err = e
            time.sleep(2.0)
    raise last_err
